# revision 1
# baseline (speedup 1.0000x reference)
"""PhasorLayer TRN2 kernel v2: bf16 GEMMs, w^T resident in SBUF (no DRAM bounce).

Math per batch row m (same as v1):
  u     = x @ [Wk|Wq|wsum]^T + [bk|bq|sum_bv]          (KQS gemm fp32, N=129)
  align = 64 - 2*sum_p sin^2((pi/2)*(tanh(uk)-tanh(uq)))
  gain  = softplus(align/64 + 0.5);  s = align*gain/64
  w     = x @ Wv^T + bv        (bf16 GEMM, fp32 accum)
  muw   = mean(w);  varw = mean(w^2) - muw^2
  inv   = rsqrt(s^2*varw + 1e-5);  a = s*inv;  c = a*muw
  out   = x + a*(w @ Wo'^T) - c*w1 + r
"""

import sys

sys.path.insert(0, "/opt/trn_rl_repo")

import math
import os
from contextlib import ExitStack

import numpy as np

import concourse.bass as bass
import concourse.mybir as mybir
import concourse.tile as tile
from concourse.alu_op_type import AluOpType
from concourse.bass_utils import run_bass_kernel_spmd
from concourse.mybir import dt
from concourse.tile_cfg import (
    BassTileBranchHintPlaceholder,
    BassTileConditionalBlock,
    BassTileCriticalSection,
    BassTileLoopBlock,
    BassTileSwitchBlock,
    TileBranchInst,
)
from concourse.vector_clock import ScopedClock

B, D, P = 8192, 4096, 64
NCORES = 8
M = B // NCORES  # 1024 batch rows per core
MT = M // 128    # 8 m-tiles
KD = D // 128    # 32 dim tiles
NB = D // 512    # 8 n-blocks
PI = math.pi
EPS = 1e-5
F32 = dt.float32
BF16 = dt.bfloat16
AF = mybir.ActivationFunctionType

_SKIP_SPLIT = (
    BassTileBranchHintPlaceholder,
    BassTileConditionalBlock,
    BassTileCriticalSection,
    BassTileLoopBlock,
    BassTileSwitchBlock,
    TileBranchInst,
)


class LegalTileContext(tile.TileContext):
    """TileContext legalized to <=1 semaphore wait per instruction."""

    def _lower_ordered_insts(self, ordered):
        for insts in ordered.values():
            out = []
            for inst in insts:
                si = getattr(inst, "sync_info", None)
                if (
                    si is not None
                    and len(si.on_wait) > 1
                    and not isinstance(inst, _SKIP_SPLIT)
                ):
                    waits = list(si.on_wait)
                    for w in waits[:-1]:
                        nop = mybir.InstNoOp(
                            name=self.nc.get_next_instruction_name(),
                            text_hint="wait_split",
                            bass_nofuse=True,
                            engine=inst.engine,
                            sync_info=mybir.SyncInfo(on_wait=[w], on_update=[]),
                        )
                        out.append(nop)
                    inst.sync_info = mybir.SyncInfo(
                        on_wait=[waits[-1]], on_update=list(si.on_update)
                    )
                out.append(inst)
            insts[:] = out
        super()._lower_ordered_insts(ordered)

    def _drain_and_barrier(self, tick_clock, wait_clock):
        drain_inst = self.nc.sync.drain()
        wait_clock.add_sem_waits(
            drain_inst.ins, ScopedClock({None: tick_clock.global_clock})
        )
        si = drain_inst.ins.sync_info
        if si is not None and len(si.on_wait) > 1:
            waits = list(si.on_wait)
            drain_inst.ins.sync_info = mybir.SyncInfo(
                on_wait=[waits[0]], on_update=list(si.on_update)
            )
            for w in waits[1:]:
                nop = self.nc.sync.nop(nofuse=True, hint="wait_split")
                nop.ins.sync_info = mybir.SyncInfo(on_wait=[w], on_update=[])
        self.nc.all_engine_barrier()
        assert self.sems is not None
        popped = self.nc._tile_sem_poison_stack.pop()
        assert popped is self._sem_poison
        self.nc.clear_and_free_semaphores(list(self.sems.allocated().values()))
        self.nc.all_engine_barrier()


def build_nc(debug=False):
    nc = bass.Bass()
    x_d = nc.declare_dram_parameter("x", [M, D], F32, isOutput=False)
    xt_d = nc.declare_dram_parameter("xt", [D, M], F32, isOutput=False)
    wvt_d = nc.declare_dram_parameter("wvt", [D, D], BF16, isOutput=False)
    w1b_d = nc.declare_dram_parameter("w1b", [128, D], BF16, isOutput=False)
    rb_d = nc.declare_dram_parameter("rb", [128, D], BF16, isOutput=False)
    wo2t_d = nc.declare_dram_parameter("wo2t", [D, D], BF16, isOutput=False)
    wkqs_d = nc.declare_dram_parameter("wkqs", [D, 129], F32, isOutput=False)
    brow_d = nc.declare_dram_parameter("brow", [128, 129], F32, isOutput=False)
    bvr_d = nc.declare_dram_parameter("bvr", [128, KD], F32, isOutput=False)
    out_d = nc.declare_dram_parameter("out", [M, D], F32, isOutput=True)

    ssq_dram = nc.dram_tensor("ssq_scr", [1, M], F32)

    with ExitStack() as ctx:
        tc = ctx.enter_context(LegalTileContext(nc))
        sb_small = ctx.enter_context(tc.tile_pool(name="small", bufs=1))

        ones_bf_t = sb_small.tile((128, 1), BF16, name="ones", tag="ones")
        nc.vector.memset(ones_bf_t[:], 1.0)
        half_t = sb_small.tile((128, 1), F32, name="half", tag="half")
        nc.vector.memset(half_t[:], 0.5)
        eps_t = sb_small.tile((128, 1), F32, name="epsb", tag="epsb")
        nc.vector.memset(eps_t[:], EPS)
        brow_t = sb_small.tile((128, 129), F32, name="browt", tag="browt")
        nc.gpsimd.dma_start(brow_t[:], brow_d[:, :])
        bvr_t = sb_small.tile((128, KD), F32, name="bvrt", tag="bvrt")
        nc.gpsimd.dma_start(bvr_t[:], bvr_d[:, :])

        def col_tile(nm):
            return sb_small.tile((128, MT), F32, name=nm, tag=nm)

        red_all = col_tile("red_all")
        align_all = col_tile("align_all")
        e1_all = col_tile("e1_all")
        gain_all = col_tile("gain_all")
        s2_all = col_tile("s2_all")
        mu_all = col_tile("mu_all")
        ssq_all = col_tile("ssq_all")
        musq_all = col_tile("musq_all")
        var_all = col_tile("var_all")
        s_all = col_tile("s_all")
        s_sq_all = col_tile("s_sq_all")
        q_all = col_tile("q_all")
        q2_all = col_tile("q2_all")
        inv_all = col_tile("inv_all")
        a_all = col_tile("a_all")
        c_all = col_tile("c_all")
        cneg_all = col_tile("cneg_all")
        acc_sb = sb_small.tile((1, M), F32, name="acc_sb", tag="acc_sb")

        # w^T residents (bf16), produced by GEMM1, consumed by GEMM2 — live
        # across both phases, no DRAM bounce.
        sb_wtb = ctx.enter_context(tc.tile_pool(name="wtbp", bufs=1))
        wtb = [
            sb_wtb.tile((128, M), BF16, name=f"wtb{k}", tag=f"wtb{k}")
            for k in range(KD)
        ]

        # epilogue constants: allocated here, loaded in small chunks during
        # GEMM1 (DMA slack there) so neither kernel start nor phase-2 wo
        # streaming queues behind the 4MB
        sb_w1r = ctx.enter_context(tc.tile_pool(name="w1rp", bufs=1))
        w1_res = sb_w1r.tile((128, D), BF16, name="w1_res", tag="w1_res")
        r_res = sb_w1r.tile((128, D), BF16, name="r_res", tag="r_res")

        # ---------------- phase 1: KQS + GEMM1 (xtb resident) ----------------
        with ExitStack() as p1:
            sb_xt = p1.enter_context(tc.tile_pool(name="xtp", bufs=1))
            xtb = [
                sb_xt.tile((128, M), BF16, name=f"xtb{j}", tag=f"xtb{j}")
                for j in range(KD)
            ]
            sb_chain = p1.enter_context(tc.tile_pool(name="chain", bufs=1))
            sb_sv = p1.enter_context(tc.tile_pool(name="sv", bufs=2))

            def align_chain():
                # serial per-m-tile chain: tanh -> sin^2 -> align -> gain
                for t in range(MT):
                    u_t = u_ts[t]
                    th_t = sb_chain.tile((128, 128), F32, name="th_t", tag="th", bufs=2)
                    nc.scalar.activation(th_t[:], u_t[:, 0:128], AF.Tanh)
                    d_t = sb_chain.tile((128, 64), F32, name="d_t", tag="d", bufs=2)
                    nc.vector.tensor_sub(d_t[:], th_t[:, 0:64], th_t[:, 64:128])
                    sn_t = sb_chain.tile((128, 64), F32, name="sn_t", tag="sn", bufs=2)
                    nc.scalar.activation(sn_t[:], d_t[:], AF.Sin, scale=PI / 2)
                    sq_t = sb_chain.tile((128, 64), F32, name="sq_t", tag="snsq", bufs=2)
                    nc.scalar.activation(
                        sq_t[:], sn_t[:], AF.Square, accum_out=red_all[:, t : t + 1]
                    )
                    nc.vector.tensor_scalar(
                        align_all[:, t : t + 1],
                        red_all[:, t : t + 1],
                        -2.0,
                        float(P),
                        AluOpType.mult,
                        AluOpType.add,
                    )
                    nc.scalar.activation(
                        e1_all[:, t : t + 1],
                        align_all[:, t : t + 1],
                        AF.Exp,
                        bias=half_t[:],
                        scale=1.0 / P,
                    )
                    nc.scalar.activation(
                        gain_all[:, t : t + 1], e1_all[:, t : t + 1], AF.Ln, bias=1.0
                    )
                    nc.vector.tensor_mul(
                        s2_all[:, t : t + 1],
                        align_all[:, t : t + 1],
                        gain_all[:, t : t + 1],
                    )
                    nc.scalar.activation(
                        mu_all[:, t : t + 1], u_t[:, 128:129], AF.Copy, scale=1.0 / D
                    )

            # --- KQS: true fp32. Stream fp32 x^T; ACT-copy to bf16 residents.
            with ExitStack() as pk:
                sb_kq = pk.enter_context(tc.tile_pool(name="kqp", bufs=1))
                sb_s1 = pk.enter_context(tc.tile_pool(name="s1", bufs=2))
                wkq_ts = []
                for j in range(KD):
                    t = sb_kq.tile((128, 129), F32, name=f"wkq{j}", tag=f"wkq{j}")
                    wkq_ts.append(t)
                nc.sync.dma_start(wkq_ts[0][:], wkqs_d[0:128, :])
                ps_kq = pk.enter_context(
                    tc.tile_pool(name="pskq", bufs=1, space="PSUM")
                )
                kq_list = [
                    ps_kq.tile((128, 129), F32, name=f"kq{t}", tag=f"kq{t}")
                    for t in range(MT)
                ]
                xs0c = [
                    sb_s1.tile((128, 256), F32, name=f"xs0c{c}", tag=f"xs0c{c}")
                    for c in range(4)
                ]
                for c in range(4):
                    nc.sync.dma_start(
                        xs0c[c][:], xt_d[0:128, c * 256 : (c + 1) * 256]
                    )
                for j in range(KD):
                    if j == 0:
                        nc.sync.dma_start(
                            wkq_ts[1][:], wkqs_d[128:256, :]
                        )
                        for t in range(MT):
                            nc.tensor.matmul(
                                kq_list[t][:],
                                xs0c[t // 2][:, (t % 2) * 128 : (t % 2 + 1) * 128],
                                wkq_ts[0][:],
                                start=True,
                                stop=False,
                            )
                        for c in range(4):
                            nc.vector.tensor_scalar(
                                xtb[0][:, c * 256 : (c + 1) * 256],
                                xs0c[c][:],
                                1.0,
                                None,
                                AluOpType.mult,
                            )
                        continue
                    xs_t = sb_s1.tile((128, M), F32, name="xs_t", tag="xs", bufs=4)
                    nc.sync.dma_start(xs_t[:], xt_d[j * 128 : (j + 1) * 128, :])
                    if j + 1 < KD:
                        nc.sync.dma_start(
                            wkq_ts[j + 1][:],
                            wkqs_d[(j + 1) * 128 : (j + 2) * 128, :],
                        )
                    nc.vector.tensor_scalar(
                        xtb[j][:], xs_t[:], 1.0, None, AluOpType.mult
                    )
                    for t in range(MT):
                        nc.tensor.matmul(
                            kq_list[t][:],
                            xs_t[:, t * 128 : (t + 1) * 128],
                            wkq_ts[j][:],
                            start=False,
                            stop=(j == KD - 1),
                        )
                # drain all PSUM kq tiles (frees banks for GEMM1's v_ps);
                # the serial align chain runs later, overlapped with GEMM1
                u_ts = []
                for t in range(MT):
                    u_t = sb_chain.tile((128, 129), F32, name=f"u{t}", tag=f"u{t}")
                    nc.vector.tensor_add(u_t[:], kq_list[t][:], brow_t[:])
                    u_ts.append(u_t)

            # --- GEMM1: w^T = Wv @ x^T + bv, bf16 operands, fp32 PSUM.
            # Output goes straight to bf16 SBUF residents (wtb).
            with ExitStack() as pg:
                ps_v = pg.enter_context(tc.tile_pool(name="psv", bufs=1, space="PSUM"))
                ps_acc = pg.enter_context(
                    tc.tile_pool(name="psacc", bufs=1, space="PSUM")
                )
                acc_ps0 = ps_acc.tile((1, 512), F32, name="acc_ps0", tag="acc0")
                acc_ps1 = ps_acc.tile((1, 512), F32, name="acc_ps1", tag="acc1")
                for kdg in range(KD // 2):
                    csl = slice((kdg % 8) * 512, (kdg % 8 + 1) * 512)
                    if kdg < 8:
                        nc.gpsimd.dma_start(w1_res[:, csl], w1b_d[:, csl])
                    else:
                        nc.gpsimd.dma_start(r_res[:, csl], rb_d[:, csl])
                    v_ps = [
                        ps_v.tile((128, 512), F32, name=f"v_ps{i}", tag=f"v{i}")
                        for i in range(4)
                    ]
                    for j in range(KD):
                        wv_t = sb_sv.tile(
                            (128, 256), BF16, name="wv_t", tag="wv", bufs=8
                        )
                        wv_eng = (
                            nc.sync
                            if j % 4 in (0, 2)
                            else (nc.scalar if j % 4 == 1 else nc.gpsimd)
                        )
                        wv_eng.dma_start(
                            wv_t[:],
                            wvt_d[j * 128 : (j + 1) * 128, kdg * 256 : (kdg + 1) * 256],
                        )
                        for i in range(4):
                            sub, half = divmod(i, 2)
                            nc.tensor.matmul(
                                v_ps[i][:],
                                wv_t[:, sub * 128 : (sub + 1) * 128],
                                xtb[j][:, half * 512 : (half + 1) * 512],
                                start=(j == 0),
                                stop=(j == KD - 1),
                            )
                    if kdg == 0:
                        # overlapped with kdg=0's matmuls on PE
                        align_chain()
                    for sub in range(2):
                        kd = kdg * 2 + sub
                        nc.vector.tensor_scalar(
                            wtb[kd][:, 0:512],
                            v_ps[sub * 2][:],
                            bvr_t[:, kd : kd + 1],
                            None,
                            AluOpType.add,
                        )
                        nc.vector.tensor_scalar(
                            wtb[kd][:, 512:1024],
                            v_ps[sub * 2 + 1][:],
                            bvr_t[:, kd : kd + 1],
                            None,
                            AluOpType.add,
                        )
                        sqw_t = sb_sv.tile(
                            (128, M), BF16, name="sqw_t", tag="sqw", bufs=2
                        )
                        nc.scalar.activation(sqw_t[:], wtb[kd][:], AF.Square)
                        nc.tensor.matmul(
                            acc_ps0[:],
                            ones_bf_t[:],
                            sqw_t[:, 0:512],
                            start=(kd == 0),
                            stop=(kd == KD - 1),
                        )
                        nc.tensor.matmul(
                            acc_ps1[:],
                            ones_bf_t[:],
                            sqw_t[:, 512:1024],
                            start=(kd == 0),
                            stop=(kd == KD - 1),
                        )

                # ssq bounce: [1, M] -> DRAM -> [128, MT] columns (Pool queue
                # so phase-2 wo loads on sync/scalar are not delayed)
                nc.scalar.copy(acc_sb[:, 0:512], acc_ps0[:])
                nc.scalar.copy(acc_sb[:, 512:1024], acc_ps1[:])
                nc.gpsimd.dma_start(ssq_dram[:, :], acc_sb[:])
                for t in range(MT):
                    nc.gpsimd.dma_start(
                        ssq_all[:, t : t + 1],
                        ssq_dram[0:1, t * 128 : (t + 1) * 128].transpose([1, 0]),
                    )

        # ---------------- scalar finalize ----------------
        nc.scalar.activation(musq_all[:], mu_all[:], AF.Square)
        nc.vector.tensor_scalar(var_all[:], ssq_all[:], 1.0 / D, None, AluOpType.mult)
        nc.vector.tensor_sub(var_all[:], var_all[:], musq_all[:])
        nc.scalar.activation(s_all[:], s2_all[:], AF.Copy, scale=1.0 / P)
        nc.scalar.activation(s_sq_all[:], s_all[:], AF.Square)
        nc.vector.tensor_mul(q_all[:], var_all[:], s_sq_all[:])
        nc.scalar.activation(q2_all[:], q_all[:], AF.Sqrt, bias=eps_t[:])
        nc.vector.reciprocal(inv_all[:], q2_all[:])
        nc.vector.tensor_mul(a_all[:], s_all[:], inv_all[:])
        nc.vector.tensor_mul(c_all[:], a_all[:], mu_all[:])
        nc.vector.tensor_scalar(cneg_all[:], c_all[:], -1.0, None, AluOpType.mult)

        # ---------------- phase 2: GEMM2 + epilogue (wtb resident) ----------------
        with ExitStack() as p2:
            sb_s2 = p2.enter_context(tc.tile_pool(name="s2", bufs=2))
            ps_p = p2.enter_context(tc.tile_pool(name="psp", bufs=1, space="PSUM"))

            for nb in range(NB):
                nsl = slice(nb * 512, (nb + 1) * 512)
                # prefetch residual x tiles for this n-block on the Pool queue
                xe_ts = []
                for mt in range(MT):
                    xe_t = sb_s2.tile(
                        (128, 512), F32, name=f"xe{mt}", tag=f"xe{mt}", bufs=2
                    )
                    nc.gpsimd.dma_start(
                        xe_t[:], x_d[mt * 128 : (mt + 1) * 128, nsl]
                    )
                    xe_ts.append(xe_t)
                # precompute xeu = x + (cneg*w1 + r) on DVE during the k-loop;
                # leaves a single STT per m-tile after the last matmul
                xeu_ts = []
                for mt in range(MT):
                    u2_t = sb_s2.tile(
                        (128, 512), F32, name=f"u2_{mt}", tag=f"u2_{mt}", bufs=1
                    )
                    nc.vector.scalar_tensor_tensor(
                        u2_t[:],
                        w1_res[:, nsl],
                        cneg_all[:, mt : mt + 1],
                        r_res[:, nsl],
                        AluOpType.mult,
                        AluOpType.add,
                    )
                    xeu_t = sb_s2.tile(
                        (128, 512), F32, name=f"xeu{mt}", tag=f"xeu{mt}", bufs=2
                    )
                    nc.vector.tensor_add(xeu_t[:], xe_ts[mt][:], u2_t[:])
                    xeu_ts.append(xeu_t)
                # two half-blocks of 4 m-tiles: half A's epilogue drains run
                # while half B's matmuls keep PE busy, so PSUM banks are free
                # by the time the next n-block (or the kernel end) needs them
                for hf in range(2):
                    mts = range(hf * 4, hf * 4 + 4)
                    p_tiles = [
                        ps_p.tile((128, 512), F32, name=f"pp{mt}", tag=f"pp{mt}")
                        for mt in mts
                    ]
                    for k in range(KD):
                        wo_t = sb_s2.tile(
                            (128, 512), BF16, name="wo_t", tag=f"wo{hf}", bufs=6
                        )
                        if nb == 0 and hf == 0 and k < 6:
                            wo_eng = nc.gpsimd
                        else:
                            wo_eng = (nc.sync, nc.scalar)[k % 2]
                        wo_eng.dma_start(
                            wo_t[:], wo2t_d[k * 128 : (k + 1) * 128, nsl]
                        )
                        for i, mt in enumerate(mts):
                            nc.tensor.matmul(
                                p_tiles[i][:],
                                wtb[k][:, mt * 128 : (mt + 1) * 128],
                                wo_t[:],
                                start=(k == 0),
                                stop=(k == KD - 1),
                            )
                    for i, mt in enumerate(mts):
                        msl = slice(mt * 128, (mt + 1) * 128)
                        oe_t = sb_s2.tile(
                            (128, 512), F32, name="oe_t", tag="oe", bufs=3
                        )
                        nc.vector.scalar_tensor_tensor(
                            oe_t[:],
                            p_tiles[i][:],
                            a_all[:, mt : mt + 1],
                            xeu_ts[mt][:],
                            AluOpType.mult,
                            AluOpType.add,
                        )
                        if nb == NB - 1 and hf == 1:
                            st_eng = (nc.gpsimd, nc.sync, nc.scalar)[mt % 3]
                        else:
                            st_eng = nc.gpsimd
                        st_eng.dma_start(out_d[msl, nsl], oe_t[:])
    return nc


def kernel(**inputs):
    import ml_dtypes

    x = np.asarray(inputs["x"], dtype=np.float32)
    Wk = np.asarray(inputs["Wk"], dtype=np.float32)
    bk = np.asarray(inputs["bk"], dtype=np.float32)
    Wq = np.asarray(inputs["Wq"], dtype=np.float32)
    bq = np.asarray(inputs["bq"], dtype=np.float32)
    Wv = np.asarray(inputs["Wv"], dtype=np.float32)
    bv = np.asarray(inputs["bv"], dtype=np.float32)
    ln_g = np.asarray(inputs["ln_g"], dtype=np.float32)
    ln_b = np.asarray(inputs["ln_b"], dtype=np.float32)
    Wo = np.asarray(inputs["Wo"], dtype=np.float32)
    bo = np.asarray(inputs["bo"], dtype=np.float32)

    Wo2T = np.ascontiguousarray((Wo * ln_g[None, :]).T)  # [k, n] = Wo'[n, k]
    w1 = Wo2T.sum(axis=0)  # [n]
    r = ln_b @ Wo.T + bo  # [n]
    WvT = np.ascontiguousarray(Wv.T)  # [j, k]
    wsum = Wv.sum(axis=0)  # [j]
    wkqs = np.ascontiguousarray(
        np.concatenate([Wk.T, Wq.T, wsum[:, None]], axis=1), dtype=np.float32
    )  # [D, 129]
    brow = np.concatenate([bk, bq, [bv.sum()]]).astype(np.float32)  # [129]
    brow_mat = np.ascontiguousarray(np.broadcast_to(brow, (128, 129)))
    w1_mat = np.ascontiguousarray(np.broadcast_to(w1, (128, D)), dtype=np.float32)
    r_mat = np.ascontiguousarray(np.broadcast_to(r, (128, D)), dtype=np.float32)
    bvr = np.ascontiguousarray(bv.reshape(KD, 128).T)  # [128, KD]
    WvT_bf = WvT.astype(ml_dtypes.bfloat16)
    Wo2T_bf = Wo2T.astype(ml_dtypes.bfloat16)

    nc = build_nc()
    in_maps = []
    for c in range(NCORES):
        xc = np.ascontiguousarray(x[c * M : (c + 1) * M])
        xtc = np.ascontiguousarray(xc.T)
        in_maps.append(
            {
                "x": xc,
                "xt": xtc,
                "wvt": WvT_bf,
                "wo2t": Wo2T_bf,
                "wkqs": wkqs,
                "brow": brow_mat,
                "bvr": bvr,
                "w1b": w1_mat.astype(ml_dtypes.bfloat16),
                "rb": r_mat.astype(ml_dtypes.bfloat16),
            }
        )
    global LAST_BUILD
    LAST_BUILD = (nc, in_maps)
    res = run_bass_kernel_spmd(nc, in_maps, core_ids=list(range(NCORES)))
    global LAST_EXEC_NS
    LAST_EXEC_NS = getattr(res, "exec_time_ns", None)
    out = np.concatenate([res.results[c]["out"] for c in range(NCORES)], axis=0)
    return out.astype(np.float32)


LAST_EXEC_NS = None
LAST_BUILD = None


if __name__ == "__main__":
    rng = np.random.default_rng(0)
    ins = {
        "x": rng.standard_normal((B, D), dtype=np.float32),
        "Wk": rng.standard_normal((P, D), dtype=np.float32) / math.sqrt(D),
        "bk": np.zeros(P, np.float32),
        "Wq": rng.standard_normal((P, D), dtype=np.float32) / math.sqrt(D),
        "bq": np.zeros(P, np.float32),
        "Wv": rng.standard_normal((D, D), dtype=np.float32) / math.sqrt(D),
        "bv": np.zeros(D, np.float32),
        "ln_g": np.ones(D, np.float32),
        "ln_b": np.zeros(D, np.float32),
        "Wo": rng.standard_normal((D, D), dtype=np.float32) / math.sqrt(D),
        "bo": np.zeros(D, np.float32),
    }
    out = kernel(**ins)
    print("out", out.shape, out.dtype, float(np.abs(out).mean()))

